# revision 24
# baseline (speedup 1.0000x reference)
"""DynamicLinear Trainium2 kernel.

Reference math (B=8192, IN=1024, OUT=1024, D=8, all fp32):
    tmp[b,d,o] = sum_i input[b,i] * weights[d,o,i]
    out[b,o]   = sum_d tmp[b,d,o] * w[b,d] + (w @ biases)[b,o]

Strategy:
  - Data parallel over batch: 8 cores x 1024 batch rows each; weights
    replicated (32 MB).
  - Host prep (layout only): inputT = input.T, weightsT = weights transposed
    to [d, i, o], wb = w @ biases (0.1% of the FLOPs).
  - Per core: for each (o-tile 512, b-tile 128, d): accumulate 8 K=128
    matmuls (fp32r -> full-speed fp32 on the PE) into a PSUM bank, then one
    DVE fused MAC: acc[b,o] = psum[b,o] * w[b,d] + acc[b,o], with acc
    initialized from wb. PE does ~17 GFLOP/core; DMA (~44 MB) and DVE
    (~8.4M MACs) overlap underneath it.
"""

import numpy as np

import concourse.bacc as bacc
import concourse.mybir as mybir
from concourse.tile import TileContext
from concourse.bass_utils import run_bass_kernel_spmd

N_CORES = 8
B, IN, OUT, D = 8192, 1024, 1024, 8
BS = B // N_CORES  # batch rows per core
P = 128            # SBUF partitions
ON = 512           # matmul moving free dim (one PSUM bank of fp32)

F32 = mybir.dt.float32
F32R = mybir.dt.float32r


def build_nc(bs=BS, in_=IN, out_=OUT, d_=D, n_wt_bufs=None, psum_bufs=4):
    nIT = in_ // P
    nBT = bs // P
    on = min(ON, out_)
    nOT = out_ // on
    if n_wt_bufs is None:
        n_wt_bufs = d_

    nc = bacc.Bacc("TRN2", target_bir_lowering=False, debug=False)
    inputT = nc.declare_dram_parameter("inputT", [in_, bs], F32, isOutput=False)
    weightsT = nc.declare_dram_parameter("weightsT", [d_, in_, out_], F32, isOutput=False)
    w = nc.declare_dram_parameter("w", [bs, d_], F32, isOutput=False)
    wb = nc.declare_dram_parameter("wb", [bs, out_], F32, isOutput=False)
    out = nc.declare_dram_parameter("out", [bs, out_], F32, isOutput=True)

    with TileContext(nc) as tc:
        with (
            tc.tile_pool(name="const", bufs=1) as const_pool,
            tc.tile_pool(name="wtpool", bufs=n_wt_bufs) as wtpool,
            tc.tile_pool(name="accpool", bufs=4) as accpool,
            tc.tile_pool(name="psumpool", bufs=psum_bufs, space="PSUM") as psumpool,
        ):
            # Resident activations: [128, nIT, bs]. float32r tiles: same bits
            # as fp32; satisfies the BIR verifier's "rounded to FP32r"
            # producer rule for fp32r matmul operands.
            inputT_sb = const_pool.tile([P, nIT, bs], F32R)
            nc.sync.dma_start(
                inputT_sb, inputT.rearrange("(it p) b -> p it b", p=P).bitcast(F32R)
            )
            # Per-partition mixing weights: [128, nBT, d_].
            w_sb = const_pool.tile([P, nBT, d_], F32)
            nc.sync.dma_start(w_sb, w.rearrange("(bt p) d -> p bt d", p=P))

            for oT in range(nOT):
                # Stream this o-slice of the transposed weights: 1 DMA per d.
                wts = []
                for dd in range(d_):
                    wt = wtpool.tile([P, nIT, on], F32R, tag="wt", name=f"wt_{oT}_{dd}")
                    src = weightsT[dd].rearrange("(it p) o -> p it o", p=P)
                    nc.sync.dma_start(wt, src[:, :, oT * on:(oT + 1) * on].bitcast(F32R))
                    wts.append(wt)

                for bT in range(nBT):
                    acc = accpool.tile([P, on], F32, tag="acc", name=f"acc_{oT}_{bT}")
                    nc.sync.dma_start(
                        acc, wb[bT * P:(bT + 1) * P, oT * on:(oT + 1) * on]
                    )
                    for dd in range(d_):
                        ps = psumpool.tile([P, on], F32, tag="ps", name=f"ps_{oT}_{bT}_{dd}")
                        for iT in range(nIT):
                            lhsT = inputT_sb[:, iT, bT * P:(bT + 1) * P]
                            rhs = wts[dd][:, iT, :]
                            nc.tensor.matmul(
                                ps,
                                lhsT,
                                rhs,
                                start=(iT == 0),
                                stop=(iT == nIT - 1),
                            )
                        # acc = psum * w[b, dd] + acc (per-partition scalar)
                        nc.vector.scalar_tensor_tensor(
                            acc,
                            ps,
                            w_sb[:, bT, dd: dd + 1],
                            acc,
                            mybir.AluOpType.mult,
                            mybir.AluOpType.add,
                        )
                    nc.sync.dma_start(
                        out[bT * P:(bT + 1) * P, oT * on:(oT + 1) * on], acc
                    )
    nc.compile()
    return nc


_nc_cache = None


def _get_nc():
    global _nc_cache
    if _nc_cache is None:
        _nc_cache = build_nc()
    return _nc_cache


def make_in_maps(input, w, weights, biases):
    input = np.ascontiguousarray(input, dtype=np.float32)
    w = np.ascontiguousarray(w, dtype=np.float32)
    weights = np.ascontiguousarray(weights, dtype=np.float32)
    biases = np.ascontiguousarray(biases, dtype=np.float32)

    inputT = np.ascontiguousarray(input.T)                       # [IN, B]
    weightsT = np.ascontiguousarray(weights.transpose(0, 2, 1))  # [D, IN, OUT]
    wb = w @ biases                                              # [B, OUT]

    in_maps = []
    for c in range(N_CORES):
        sl = slice(c * BS, (c + 1) * BS)
        in_maps.append({
            "inputT": np.ascontiguousarray(inputT[:, sl]),
            "weightsT": weightsT,
            "w": np.ascontiguousarray(w[sl]),
            "wb": np.ascontiguousarray(wb[sl]),
        })
    return in_maps


def kernel(input, w, weights, biases):
    in_maps = make_in_maps(input, w, weights, biases)
    res = run_bass_kernel_spmd(_get_nc(), in_maps, list(range(N_CORES)))
    return np.concatenate(
        [res.results[c]["out"] for c in range(N_CORES)], axis=0
    ).astype(np.float32)


if __name__ == "__main__":
    rng = np.random.default_rng(0)
    inputs = {
        "input": rng.standard_normal((B, IN), dtype=np.float32),
        "w": rng.random((B, D), dtype=np.float32),
        "weights": ((rng.random((D, OUT, IN), dtype=np.float32) - 0.5) / 16.0),
        "biases": ((rng.random((D, OUT), dtype=np.float32) - 0.5) / 16.0),
    }
    got = kernel(**inputs)
    tmp = np.einsum("bi,doi->bdo", inputs["input"], inputs["weights"])
    want = np.einsum("bdo,bd->bo", tmp, inputs["w"]) + inputs["w"] @ inputs["biases"]
    err = np.abs(got - want).max() / np.abs(want).max()
    print("rel err:", err)


# revision 27
# speedup vs baseline: 1.2162x; 1.2162x over previous
"""DynamicLinear Trainium2 kernel.

Reference math (B=8192, IN=1024, OUT=1024, D=8, all fp32):
    tmp[b,d,o] = sum_i input[b,i] * weights[d,o,i]
    out[b,o]   = sum_d tmp[b,d,o] * w[b,d] + (w @ biases)[b,o]

Strategy:
  - Data parallel over batch: 8 cores x 1024 batch rows each; weights
    replicated (32 MB).
  - Host prep (layout only): inputT = input.T, weightsT = weights transposed
    to [d, i, o], wb = w @ biases (0.1% of the FLOPs).
  - Per core: for each (o-tile 512, b-tile 128, d): accumulate 8 K=128
    matmuls (fp32r -> full-speed fp32 on the PE) into a PSUM bank, then one
    DVE fused MAC: acc[b,o] = psum[b,o] * w[b,d] + acc[b,o], with acc
    initialized from wb. PE does ~17 GFLOP/core; DMA (~44 MB) and DVE
    (~8.4M MACs) overlap underneath it.
"""

import numpy as np

import concourse.bacc as bacc
import concourse.mybir as mybir
from concourse.tile import TileContext
from concourse.bass_utils import run_bass_kernel_spmd

N_CORES = 8
B, IN, OUT, D = 8192, 1024, 1024, 8
BS = B // N_CORES  # batch rows per core
P = 128            # SBUF partitions
ON = 512           # matmul moving free dim (one PSUM bank of fp32)

F32 = mybir.dt.float32
F32R = mybir.dt.float32r


def build_nc(bs=BS, in_=IN, out_=OUT, d_=D, n_wt_bufs=4, psum_bufs=4):
    nIT = in_ // P
    nBT = bs // P
    on = min(ON, out_)
    nOT = out_ // on
    n_wt_bufs = min(n_wt_bufs, d_)

    nc = bacc.Bacc("TRN2", target_bir_lowering=False, debug=False)
    inputT = nc.declare_dram_parameter("inputT", [in_, bs], F32, isOutput=False)
    weightsT = nc.declare_dram_parameter("weightsT", [d_, in_, out_], F32, isOutput=False)
    w = nc.declare_dram_parameter("w", [bs, d_], F32, isOutput=False)
    wb = nc.declare_dram_parameter("wb", [bs, out_], F32, isOutput=False)
    out = nc.declare_dram_parameter("out", [bs, out_], F32, isOutput=True)

    with TileContext(nc) as tc:
        with (
            tc.tile_pool(name="const", bufs=1) as const_pool,
            tc.tile_pool(name="wtpool", bufs=n_wt_bufs) as wtpool,
            tc.tile_pool(name="accpool", bufs=10) as accpool,
            tc.tile_pool(name="psumpool", bufs=psum_bufs, space="PSUM") as psumpool,
        ):
            # Resident activations: [128, nIT, bs]. float32r tiles: same bits
            # as fp32; satisfies the BIR verifier's "rounded to FP32r"
            # producer rule for fp32r matmul operands.
            inputT_sb = const_pool.tile([P, nIT, bs], F32R)
            nc.sync.dma_start(
                inputT_sb, inputT.rearrange("(it p) b -> p it b", p=P).bitcast(F32R)
            )
            # Per-partition mixing weights: [128, nBT, d_].
            w_sb = const_pool.tile([P, nBT, d_], F32)
            nc.sync.dma_start(w_sb, w.rearrange("(bt p) d -> p bt d", p=P))

            for oT in range(nOT):
                # Per-pass output accumulators, initialized with the bias term.
                accs = []
                for bT in range(nBT):
                    acc = accpool.tile(
                        [P, on], F32, tag="acc", name=f"acc_{oT}_{bT}"
                    )
                    nc.sync.dma_start(
                        acc, wb[bT * P:(bT + 1) * P, oT * on:(oT + 1) * on]
                    )
                    accs.append(acc)

                # d OUTER, bT inner: each streamed weights tile (2 MB, ~6 us
                # DMA) is consumed by 8 back-to-back matmul groups (~16 us of
                # PE work), so the DMA stays ahead and the PE never stalls
                # (stalling also re-throttles the PE clock to 1.2 GHz).
                for dd in range(d_):
                    wt = wtpool.tile([P, nIT, on], F32R, tag="wt", name=f"wt_{oT}_{dd}")
                    src = weightsT[dd].rearrange("(it p) o -> p it o", p=P)
                    nc.sync.dma_start(wt, src[:, :, oT * on:(oT + 1) * on].bitcast(F32R))
                    for bT in range(nBT):
                        ps = psumpool.tile([P, on], F32, tag="ps", name=f"ps_{oT}_{bT}_{dd}")
                        for iT in range(nIT):
                            lhsT = inputT_sb[:, iT, bT * P:(bT + 1) * P]
                            rhs = wt[:, iT, :]
                            nc.tensor.matmul(
                                ps,
                                lhsT,
                                rhs,
                                start=(iT == 0),
                                stop=(iT == nIT - 1),
                            )
                        # acc = psum * w[b, dd] + acc (per-partition scalar)
                        nc.vector.scalar_tensor_tensor(
                            accs[bT],
                            ps,
                            w_sb[:, bT, dd: dd + 1],
                            accs[bT],
                            mybir.AluOpType.mult,
                            mybir.AluOpType.add,
                        )
                for bT in range(nBT):
                    nc.sync.dma_start(
                        out[bT * P:(bT + 1) * P, oT * on:(oT + 1) * on], accs[bT]
                    )
    nc.compile()
    return nc


_nc_cache = None


def _get_nc():
    global _nc_cache
    if _nc_cache is None:
        _nc_cache = build_nc()
    return _nc_cache


def make_in_maps(input, w, weights, biases):
    input = np.ascontiguousarray(input, dtype=np.float32)
    w = np.ascontiguousarray(w, dtype=np.float32)
    weights = np.ascontiguousarray(weights, dtype=np.float32)
    biases = np.ascontiguousarray(biases, dtype=np.float32)

    inputT = np.ascontiguousarray(input.T)                       # [IN, B]
    weightsT = np.ascontiguousarray(weights.transpose(0, 2, 1))  # [D, IN, OUT]
    wb = w @ biases                                              # [B, OUT]

    in_maps = []
    for c in range(N_CORES):
        sl = slice(c * BS, (c + 1) * BS)
        in_maps.append({
            "inputT": np.ascontiguousarray(inputT[:, sl]),
            "weightsT": weightsT,
            "w": np.ascontiguousarray(w[sl]),
            "wb": np.ascontiguousarray(wb[sl]),
        })
    return in_maps


def kernel(input, w, weights, biases):
    in_maps = make_in_maps(input, w, weights, biases)
    res = run_bass_kernel_spmd(_get_nc(), in_maps, list(range(N_CORES)))
    return np.concatenate(
        [res.results[c]["out"] for c in range(N_CORES)], axis=0
    ).astype(np.float32)


if __name__ == "__main__":
    rng = np.random.default_rng(0)
    inputs = {
        "input": rng.standard_normal((B, IN), dtype=np.float32),
        "w": rng.random((B, D), dtype=np.float32),
        "weights": ((rng.random((D, OUT, IN), dtype=np.float32) - 0.5) / 16.0),
        "biases": ((rng.random((D, OUT), dtype=np.float32) - 0.5) / 16.0),
    }
    got = kernel(**inputs)
    tmp = np.einsum("bi,doi->bdo", inputs["input"], inputs["weights"])
    want = np.einsum("bdo,bd->bo", tmp, inputs["w"]) + inputs["w"] @ inputs["biases"]
    err = np.abs(got - want).max() / np.abs(want).max()
    print("rel err:", err)


# revision 30
# speedup vs baseline: 1.2559x; 1.0327x over previous
"""DynamicLinear Trainium2 kernel.

Reference math (B=8192, IN=1024, OUT=1024, D=8, all fp32):
    tmp[b,d,o] = sum_i input[b,i] * weights[d,o,i]
    out[b,o]   = sum_d tmp[b,d,o] * w[b,d] + (w @ biases)[b,o]

Strategy:
  - Data parallel over batch: 8 cores x 1024 batch rows each; weights
    replicated (32 MB).
  - Host prep (layout only): inputT = input.T, weightsT = weights transposed
    to [d, i, o], wb = w @ biases (0.1% of the FLOPs).
  - Per core: for each (o-tile 512, b-tile 128, d): accumulate 8 K=128
    matmuls (fp32r -> full-speed fp32 on the PE) into a PSUM bank, then one
    DVE fused MAC: acc[b,o] = psum[b,o] * w[b,d] + acc[b,o], with acc
    initialized from wb. PE does ~17 GFLOP/core; DMA (~44 MB) and DVE
    (~8.4M MACs) overlap underneath it.
"""

import numpy as np

import concourse.bacc as bacc
import concourse.mybir as mybir
from concourse.tile import TileContext
from concourse.bass_utils import run_bass_kernel_spmd

N_CORES = 8
B, IN, OUT, D = 8192, 1024, 1024, 8
BS = B // N_CORES  # batch rows per core
P = 128            # SBUF partitions
ON = 512           # matmul moving free dim (one PSUM bank of fp32)

F32 = mybir.dt.float32
F32R = mybir.dt.float32r


def build_nc(bs=BS, in_=IN, out_=OUT, d_=D, n_wt_bufs=4, psum_bufs=8):
    nIT = in_ // P
    nBT = bs // P
    on = min(ON, out_)
    nOT = out_ // on
    n_wt_bufs = min(n_wt_bufs, d_)

    nc = bacc.Bacc("TRN2", target_bir_lowering=False, debug=False)
    inputT = nc.declare_dram_parameter("inputT", [in_, bs], F32, isOutput=False)
    weightsT = nc.declare_dram_parameter("weightsT", [d_, in_, out_], F32, isOutput=False)
    w = nc.declare_dram_parameter("w", [bs, d_], F32, isOutput=False)
    wb = nc.declare_dram_parameter("wb", [bs, out_], F32, isOutput=False)
    out = nc.declare_dram_parameter("out", [bs, out_], F32, isOutput=True)

    with TileContext(nc) as tc:
        with (
            tc.tile_pool(name="const", bufs=1) as const_pool,
            tc.tile_pool(name="wtpool", bufs=n_wt_bufs) as wtpool,
            tc.tile_pool(name="accpool", bufs=10) as accpool,
            tc.tile_pool(name="psumpool", bufs=psum_bufs, space="PSUM") as psumpool,
        ):
            # Resident activations: [128, nIT, bs]. float32r tiles: same bits
            # as fp32; satisfies the BIR verifier's "rounded to FP32r"
            # producer rule for fp32r matmul operands. DMA'd per i-tile slice
            # (interleaved with the first weights tile's slices below) so the
            # first matmuls start after ~0.75 MB instead of 6 MB.
            inputT_sb = const_pool.tile([P, nIT, bs], F32R)
            inputT_src = inputT.rearrange("(it p) b -> p it b", p=P).bitcast(F32R)
            # Per-partition mixing weights: [128, nBT, d_].
            w_sb = const_pool.tile([P, nBT, d_], F32)
            nc.sync.dma_start(w_sb, w.rearrange("(bt p) d -> p bt d", p=P))

            for oT in range(nOT):
                # d OUTER: each streamed weights tile (2 MB, ~6 us DMA) is
                # consumed by ~16 us of PE work, so the DMA stays ahead and
                # the PE never stalls (stalling also re-throttles the PE
                # clock to 1.2 GHz). iT OUTER within a block, with all 8
                # b-tile accumulation groups open across the 8 PSUM banks:
                # at kernel start the PE can begin as soon as the first
                # (inputT slice, weights slice) pair lands.
                accs = []
                for dd in range(d_):
                    wt = wtpool.tile([P, nIT, on], F32R, tag="wt", name=f"wt_{oT}_{dd}")
                    src = weightsT[dd].rearrange("(it p) o -> p it o", p=P)
                    src = src[:, :, oT * on:(oT + 1) * on].bitcast(F32R)
                    if oT == 0 and dd == 0:
                        # Cold start: interleave inputT and first-weights
                        # slices so matmuls can chase the DMA stream.
                        for iT in range(nIT):
                            nc.sync.dma_start(
                                inputT_sb[:, iT, :], inputT_src[:, iT, :]
                            )
                            nc.sync.dma_start(wt[:, iT, :], src[:, iT, :])
                    else:
                        nc.sync.dma_start(wt, src)
                    if dd == 0:
                        # Pass accumulators, initialized with the bias term.
                        # Emitted after the critical-path DMAs above.
                        for bT in range(nBT):
                            acc = accpool.tile(
                                [P, on], F32, tag="acc", name=f"acc_{oT}_{bT}"
                            )
                            nc.sync.dma_start(
                                acc, wb[bT * P:(bT + 1) * P, oT * on:(oT + 1) * on]
                            )
                            accs.append(acc)
                    pss = [
                        psumpool.tile([P, on], F32, tag="ps", name=f"ps_{oT}_{dd}_{bT}")
                        for bT in range(nBT)
                    ]
                    for iT in range(nIT):
                        for bT in range(nBT):
                            lhsT = inputT_sb[:, iT, bT * P:(bT + 1) * P]
                            nc.tensor.matmul(
                                pss[bT],
                                lhsT,
                                wt[:, iT, :],
                                start=(iT == 0),
                                stop=(iT == nIT - 1),
                            )
                    for bT in range(nBT):
                        # acc = psum * w[b, dd] + acc (per-partition scalar)
                        nc.vector.scalar_tensor_tensor(
                            accs[bT],
                            pss[bT],
                            w_sb[:, bT, dd: dd + 1],
                            accs[bT],
                            mybir.AluOpType.mult,
                            mybir.AluOpType.add,
                        )
                for bT in range(nBT):
                    nc.sync.dma_start(
                        out[bT * P:(bT + 1) * P, oT * on:(oT + 1) * on], accs[bT]
                    )
    nc.compile()
    return nc


_nc_cache = None


def _get_nc():
    global _nc_cache
    if _nc_cache is None:
        _nc_cache = build_nc()
    return _nc_cache


def make_in_maps(input, w, weights, biases):
    input = np.ascontiguousarray(input, dtype=np.float32)
    w = np.ascontiguousarray(w, dtype=np.float32)
    weights = np.ascontiguousarray(weights, dtype=np.float32)
    biases = np.ascontiguousarray(biases, dtype=np.float32)

    inputT = np.ascontiguousarray(input.T)                       # [IN, B]
    weightsT = np.ascontiguousarray(weights.transpose(0, 2, 1))  # [D, IN, OUT]
    wb = w @ biases                                              # [B, OUT]

    in_maps = []
    for c in range(N_CORES):
        sl = slice(c * BS, (c + 1) * BS)
        in_maps.append({
            "inputT": np.ascontiguousarray(inputT[:, sl]),
            "weightsT": weightsT,
            "w": np.ascontiguousarray(w[sl]),
            "wb": np.ascontiguousarray(wb[sl]),
        })
    return in_maps


def kernel(input, w, weights, biases):
    in_maps = make_in_maps(input, w, weights, biases)
    res = run_bass_kernel_spmd(_get_nc(), in_maps, list(range(N_CORES)))
    return np.concatenate(
        [res.results[c]["out"] for c in range(N_CORES)], axis=0
    ).astype(np.float32)


if __name__ == "__main__":
    rng = np.random.default_rng(0)
    inputs = {
        "input": rng.standard_normal((B, IN), dtype=np.float32),
        "w": rng.random((B, D), dtype=np.float32),
        "weights": ((rng.random((D, OUT, IN), dtype=np.float32) - 0.5) / 16.0),
        "biases": ((rng.random((D, OUT), dtype=np.float32) - 0.5) / 16.0),
    }
    got = kernel(**inputs)
    tmp = np.einsum("bi,doi->bdo", inputs["input"], inputs["weights"])
    want = np.einsum("bdo,bd->bo", tmp, inputs["w"]) + inputs["w"] @ inputs["biases"]
    err = np.abs(got - want).max() / np.abs(want).max()
    print("rel err:", err)


# revision 39
# speedup vs baseline: 1.2826x; 1.0212x over previous
"""DynamicLinear Trainium2 kernel.

Reference math (B=8192, IN=1024, OUT=1024, D=8, all fp32):
    tmp[b,d,o] = sum_i input[b,i] * weights[d,o,i]
    out[b,o]   = sum_d tmp[b,d,o] * w[b,d] + (w @ biases)[b,o]

Strategy:
  - Data parallel over batch: 8 cores x 1024 batch rows each; weights
    replicated (32 MB).
  - Host prep (layout only): inputT = input.T, weightsT = weights transposed
    to [d, i, o], wb = w @ biases (0.1% of the FLOPs).
  - Per core: for each (o-tile 512, b-tile 128, d): accumulate 8 K=128
    matmuls (fp32r -> full-speed fp32 on the PE) into a PSUM bank, then one
    DVE fused MAC: acc[b,o] = psum[b,o] * w[b,d] + acc[b,o], with acc
    initialized from wb. PE does ~17 GFLOP/core; DMA (~44 MB) and DVE
    (~8.4M MACs) overlap underneath it.
"""

import numpy as np

import concourse.bacc as bacc
import concourse.mybir as mybir
from concourse.tile import TileContext
from concourse.bass_utils import run_bass_kernel_spmd

N_CORES = 8
B, IN, OUT, D = 8192, 1024, 1024, 8
BS = B // N_CORES  # batch rows per core
P = 128            # SBUF partitions
ON = 512           # matmul moving free dim (one PSUM bank of fp32)

F32 = mybir.dt.float32
F32R = mybir.dt.float32r


def build_nc(bs=BS, in_=IN, out_=OUT, d_=D, n_wt_bufs=4, psum_bufs=8):
    nIT = in_ // P
    nBT = bs // P
    on = min(ON, out_)
    nOT = out_ // on
    n_wt_bufs = min(n_wt_bufs, d_)

    nc = bacc.Bacc("TRN2", target_bir_lowering=False, debug=False)
    inputT = nc.declare_dram_parameter("inputT", [in_, bs], F32, isOutput=False)
    weightsT = nc.declare_dram_parameter("weightsT", [d_, in_, out_], F32, isOutput=False)
    w = nc.declare_dram_parameter("w", [bs, d_], F32, isOutput=False)
    out = nc.declare_dram_parameter("out", [bs, out_], F32, isOutput=True)

    with TileContext(nc) as tc:
        with (
            tc.tile_pool(name="const", bufs=1) as const_pool,
            tc.tile_pool(name="wtpool", bufs=n_wt_bufs) as wtpool,
            tc.tile_pool(name="accpool", bufs=10) as accpool,
            tc.tile_pool(name="psumpool", bufs=psum_bufs, space="PSUM") as psumpool,
        ):
            # Resident activations: [128, nIT, bs]. float32r tiles: same bits
            # as fp32; satisfies the BIR verifier's "rounded to FP32r"
            # producer rule for fp32r matmul operands. DMA'd per i-tile slice
            # (interleaved with the first weights tile's slices below) so the
            # first matmuls start after ~0.75 MB instead of 6 MB.
            inputT_sb = const_pool.tile([P, nIT, bs], F32R)
            inputT_src = inputT.rearrange("(it p) b -> p it b", p=P).bitcast(F32R)
            # Per-partition mixing weights: [128, nBT, d_].
            w_sb = const_pool.tile([P, nBT, d_], F32)
            nc.sync.dma_start(w_sb, w.rearrange("(bt p) d -> p bt d", p=P))

            def dma_wt(oT, dd, cold=False):
                wt = wtpool.tile([P, nIT, on], F32R, tag="wt", name=f"wt_{oT}_{dd}")
                src = weightsT[dd].rearrange("(it p) o -> p it o", p=P)
                src = src[:, :, oT * on:(oT + 1) * on].bitcast(F32R)
                if cold:
                    # Cold start: interleave inputT and first-weights slices
                    # so matmuls can chase the DMA stream.
                    for iT in range(nIT):
                        nc.sync.dma_start(inputT_sb[:, iT, :], inputT_src[:, iT, :])
                        nc.sync.dma_start(wt[:, iT, :], src[:, iT, :])
                else:
                    nc.sync.dma_start(wt, src)
                return wt

            # d OUTER: each streamed weights tile (2 MB, ~6 us DMA) covers
            # ~16 us of PE work, so the DMA stays ahead and the PE never
            # stalls (stalling also re-throttles the PE clock to 1.2 GHz).
            # Each block prefetches the NEXT block's weights DMA before its
            # own matmuls so the SP queue keeps one tile of lookahead.
            # iT OUTER within a block, with all 8 b-tile accumulation groups
            # open across the 8 PSUM banks: at kernel start the PE begins as
            # soon as the first (inputT slice, weights slice) pair lands.
            blocks = [(oT, dd) for oT in range(nOT) for dd in range(d_)]
            wt_next = dma_wt(*blocks[0], cold=True)
            all_accs = {}

            def mac(accs, pss, oT, dd, bT):
                if dd == 0:
                    # First d: acc = psum * w[b, 0] — no accumulate read, so
                    # the MAC (and the PSUM-slot release the next block
                    # waits on) has no DMA dependency. The bias term is
                    # added on the host.
                    nc.vector.tensor_scalar_mul(
                        accs[bT], pss[bT], w_sb[:, bT, 0:1]
                    )
                else:
                    # acc = psum * w[b, dd] + acc (per-partition scalar)
                    nc.vector.scalar_tensor_tensor(
                        accs[bT],
                        pss[bT],
                        w_sb[:, bT, dd: dd + 1],
                        accs[bT],
                        mybir.AluOpType.mult,
                        mybir.AluOpType.add,
                    )

            for bi, (oT, dd) in enumerate(blocks):
                wt = wt_next
                if bi + 1 < len(blocks):
                    wt_next = dma_wt(*blocks[bi + 1])
                if dd == 0:
                    all_accs[oT] = [
                        accpool.tile([P, on], F32, tag="acc", name=f"acc_{oT}_{bT}")
                        for bT in range(nBT)
                    ]
                accs = all_accs[oT]
                last_block = bi == len(blocks) - 1
                if not last_block:
                    pss = [
                        psumpool.tile([P, on], F32, tag="ps", name=f"ps_{oT}_{dd}_{bT}")
                        for bT in range(nBT)
                    ]
                    for iT in range(nIT):
                        for bT in range(nBT):
                            lhsT = inputT_sb[:, iT, bT * P:(bT + 1) * P]
                            nc.tensor.matmul(
                                pss[bT],
                                lhsT,
                                wt[:, iT, :],
                                start=(iT == 0),
                                stop=(iT == nIT - 1),
                            )
                    for bT in range(nBT):
                        mac(accs, pss, oT, dd, bT)
                else:
                    # Final block: bT-ordered groups so each MAC + store
                    # trails its group under the next group's matmuls,
                    # instead of all 8 being exposed after the last matmul.
                    pss = [
                        psumpool.tile([P, on], F32, tag="ps", name=f"ps_{oT}_{dd}_{bT}")
                        for bT in range(nBT)
                    ]
                    for bT in range(nBT):
                        for iT in range(nIT):
                            lhsT = inputT_sb[:, iT, bT * P:(bT + 1) * P]
                            nc.tensor.matmul(
                                pss[bT],
                                lhsT,
                                wt[:, iT, :],
                                start=(iT == 0),
                                stop=(iT == nIT - 1),
                            )
                        mac(accs, pss, oT, dd, bT)
                        nc.sync.dma_start(
                            out[bT * P:(bT + 1) * P, oT * on:(oT + 1) * on],
                            accs[bT],
                        )
                if dd == d_ - 1 and not last_block:
                    for bT in range(nBT):
                        nc.sync.dma_start(
                            out[bT * P:(bT + 1) * P, oT * on:(oT + 1) * on],
                            accs[bT],
                        )
    nc.compile()
    return nc


_nc_cache = None


def _get_nc():
    global _nc_cache
    if _nc_cache is None:
        _nc_cache = build_nc()
    return _nc_cache


def make_in_maps(input, w, weights, biases):
    input = np.ascontiguousarray(input, dtype=np.float32)
    w = np.ascontiguousarray(w, dtype=np.float32)
    weights = np.ascontiguousarray(weights, dtype=np.float32)
    biases = np.ascontiguousarray(biases, dtype=np.float32)

    inputT = np.ascontiguousarray(input.T)                       # [IN, B]
    weightsT = np.ascontiguousarray(weights.transpose(0, 2, 1))  # [D, IN, OUT]

    in_maps = []
    for c in range(N_CORES):
        sl = slice(c * BS, (c + 1) * BS)
        in_maps.append({
            "inputT": np.ascontiguousarray(inputT[:, sl]),
            "weightsT": weightsT,
            "w": np.ascontiguousarray(w[sl]),
        })
    return in_maps


def kernel(input, w, weights, biases):
    in_maps = make_in_maps(input, w, weights, biases)
    res = run_bass_kernel_spmd(_get_nc(), in_maps, list(range(N_CORES)))
    dev = np.concatenate(
        [res.results[c]["out"] for c in range(N_CORES)], axis=0
    ).astype(np.float32)
    # Bias term (0.1% of the FLOPs) added on host.
    wb = np.asarray(w, dtype=np.float32) @ np.asarray(biases, dtype=np.float32)
    return dev + wb


if __name__ == "__main__":
    rng = np.random.default_rng(0)
    inputs = {
        "input": rng.standard_normal((B, IN), dtype=np.float32),
        "w": rng.random((B, D), dtype=np.float32),
        "weights": ((rng.random((D, OUT, IN), dtype=np.float32) - 0.5) / 16.0),
        "biases": ((rng.random((D, OUT), dtype=np.float32) - 0.5) / 16.0),
    }
    got = kernel(**inputs)
    tmp = np.einsum("bi,doi->bdo", inputs["input"], inputs["weights"])
    want = np.einsum("bdo,bd->bo", tmp, inputs["w"]) + inputs["w"] @ inputs["biases"]
    err = np.abs(got - want).max() / np.abs(want).max()
    print("rel err:", err)


# revision 46
# speedup vs baseline: 1.3203x; 1.0295x over previous
"""DynamicLinear Trainium2 kernel.

Reference math (B=8192, IN=1024, OUT=1024, D=8, all fp32):
    tmp[b,d,o] = sum_i input[b,i] * weights[d,o,i]
    out[b,o]   = sum_d tmp[b,d,o] * w[b,d] + (w @ biases)[b,o]

Strategy:
  - Data parallel over batch: 8 cores x 1024 batch rows each; weights
    replicated (32 MB).
  - Host prep (layout only): inputT = input.T, weightsT = weights transposed
    to [d, i, o], wb = w @ biases (0.1% of the FLOPs).
  - Per core: for each (o-tile 512, b-tile 128, d): accumulate 8 K=128
    matmuls (fp32r -> full-speed fp32 on the PE) into a PSUM bank, then one
    DVE fused MAC: acc[b,o] = psum[b,o] * w[b,d] + acc[b,o], with acc
    initialized from wb. PE does ~17 GFLOP/core; DMA (~44 MB) and DVE
    (~8.4M MACs) overlap underneath it.
"""

import numpy as np

import concourse.bacc as bacc
import concourse.mybir as mybir
from concourse.tile import TileContext
from concourse.bass_utils import run_bass_kernel_spmd

N_CORES = 8
B, IN, OUT, D = 8192, 1024, 1024, 8
BS = B // N_CORES  # batch rows per core
P = 128            # SBUF partitions
ON = 512           # matmul moving free dim (one PSUM bank of fp32)

F32 = mybir.dt.float32
F32R = mybir.dt.float32r


def build_nc(bs=BS, in_=IN, out_=OUT, d_=D, n_wt_bufs=4, psum_bufs=8):
    nIT = in_ // P
    nBT = bs // P
    on = min(ON, out_)
    nOT = out_ // on
    n_wt_bufs = min(n_wt_bufs, d_)

    nc = bacc.Bacc("TRN2", target_bir_lowering=False, debug=False)
    inputT = nc.declare_dram_parameter("inputT", [in_, bs], F32, isOutput=False)
    weightsT = nc.declare_dram_parameter("weightsT", [d_, in_, out_], F32, isOutput=False)
    w = nc.declare_dram_parameter("w", [bs, d_], F32, isOutput=False)
    out = nc.declare_dram_parameter("out", [bs, out_], F32, isOutput=True)

    with TileContext(nc) as tc:
        with (
            tc.tile_pool(name="const", bufs=1) as const_pool,
            tc.tile_pool(name="wtpool", bufs=n_wt_bufs) as wtpool,
            tc.tile_pool(name="accpool", bufs=10) as accpool,
            tc.tile_pool(name="psumpool", bufs=psum_bufs, space="PSUM") as psumpool,
        ):
            # Resident activations: [128, nIT, bs]. float32r tiles: same bits
            # as fp32; satisfies the BIR verifier's "rounded to FP32r"
            # producer rule for fp32r matmul operands. DMA'd per i-tile slice
            # (interleaved with the first weights tile's slices below) so the
            # first matmuls start after ~0.75 MB instead of 6 MB.
            inputT_sb = const_pool.tile([P, nIT, bs], F32R)
            inputT_src = inputT.rearrange("(it p) b -> p it b", p=P).bitcast(F32R)
            # Per-partition mixing weights: [128, nBT, d_] (DMA'd below,
            # after the cold-start critical path — first needed ~30 us in).
            w_sb = const_pool.tile([P, nBT, d_], F32)

            def dma_wt(oT, dd, cold=False):
                # Per-iT-slice DMAs (8 completion events per tile): a block's
                # first matmul waits on a 256 KB slice, not the whole 2 MB.
                wt = wtpool.tile([P, nIT, on], F32R, tag="wt", name=f"wt_{oT}_{dd}")
                src = weightsT[dd].rearrange("(it p) o -> p it o", p=P)
                src = src[:, :, oT * on:(oT + 1) * on].bitcast(F32R)
                for iT in range(nIT):
                    if cold and iT == 0:
                        # Cold start, first slice pair: the first b-column
                        # block of inputT and the first weights slice go
                        # first, so matmul #1 waits on ~320 KB; the rest of
                        # the b-columns follow.
                        nc.sync.dma_start(
                            inputT_sb[:, 0, 0:P], inputT_src[:, 0, 0:P]
                        )
                        nc.sync.dma_start(wt[:, 0, :], src[:, 0, :])
                        nc.sync.dma_start(
                            inputT_sb[:, 0, P:bs], inputT_src[:, 0, P:bs]
                        )
                        continue
                    if cold:
                        # Interleave inputT and first-weights slices so
                        # matmuls can chase the DMA stream.
                        nc.sync.dma_start(inputT_sb[:, iT, :], inputT_src[:, iT, :])
                    nc.sync.dma_start(wt[:, iT, :], src[:, iT, :])
                return wt

            # d OUTER: each streamed weights tile (2 MB, ~6 us DMA) covers
            # ~16 us of PE work, so the DMA stays ahead and the PE never
            # stalls (stalling also re-throttles the PE clock to 1.2 GHz).
            # Each block prefetches the NEXT block's weights DMA before its
            # own matmuls so the SP queue keeps one tile of lookahead.
            # iT OUTER within a block, with all 8 b-tile accumulation groups
            # open across the 8 PSUM banks: at kernel start the PE begins as
            # soon as the first (inputT slice, weights slice) pair lands.
            blocks = [(oT, dd) for oT in range(nOT) for dd in range(d_)]
            wt_next = dma_wt(*blocks[0], cold=True)
            nc.sync.dma_start(w_sb, w.rearrange("(bt p) d -> p bt d", p=P))
            all_accs = {}

            def mac(accs, pss, oT, dd, bT):
                if dd == 0:
                    # First d: acc = psum * w[b, 0] — no accumulate read, so
                    # the MAC (and the PSUM-slot release the next block
                    # waits on) has no DMA dependency. The bias term is
                    # added on the host.
                    nc.vector.tensor_scalar_mul(
                        accs[bT], pss[bT], w_sb[:, bT, 0:1]
                    )
                else:
                    # acc = psum * w[b, dd] + acc (per-partition scalar)
                    nc.vector.scalar_tensor_tensor(
                        accs[bT],
                        pss[bT],
                        w_sb[:, bT, dd: dd + 1],
                        accs[bT],
                        mybir.AluOpType.mult,
                        mybir.AluOpType.add,
                    )

            for bi, (oT, dd) in enumerate(blocks):
                wt = wt_next
                if bi + 1 < len(blocks):
                    wt_next = dma_wt(*blocks[bi + 1])
                if dd == 0:
                    all_accs[oT] = [
                        accpool.tile([P, on], F32, tag="acc", name=f"acc_{oT}_{bT}")
                        for bT in range(nBT)
                    ]
                accs = all_accs[oT]
                last_block = bi == len(blocks) - 1
                if not last_block:
                    pss = [
                        psumpool.tile([P, on], F32, tag="ps", name=f"ps_{oT}_{dd}_{bT}")
                        for bT in range(nBT)
                    ]
                    for iT in range(nIT):
                        for bT in range(nBT):
                            lhsT = inputT_sb[:, iT, bT * P:(bT + 1) * P]
                            nc.tensor.matmul(
                                pss[bT],
                                lhsT,
                                wt[:, iT, :],
                                start=(iT == 0),
                                stop=(iT == nIT - 1),
                            )
                    for bT in range(nBT):
                        mac(accs, pss, oT, dd, bT)
                else:
                    # Final block: bT-ordered groups so each MAC + store
                    # trails its group under the next group's matmuls,
                    # instead of all 8 being exposed after the last matmul.
                    pss = [
                        psumpool.tile([P, on], F32, tag="ps", name=f"ps_{oT}_{dd}_{bT}")
                        for bT in range(nBT)
                    ]
                    for bT in range(nBT):
                        for iT in range(nIT):
                            lhsT = inputT_sb[:, iT, bT * P:(bT + 1) * P]
                            nc.tensor.matmul(
                                pss[bT],
                                lhsT,
                                wt[:, iT, :],
                                start=(iT == 0),
                                stop=(iT == nIT - 1),
                            )
                        mac(accs, pss, oT, dd, bT)
                        nc.sync.dma_start(
                            out[bT * P:(bT + 1) * P, oT * on:(oT + 1) * on],
                            accs[bT],
                        )
                if dd == d_ - 1 and not last_block:
                    for bT in range(nBT):
                        nc.sync.dma_start(
                            out[bT * P:(bT + 1) * P, oT * on:(oT + 1) * on],
                            accs[bT],
                        )
    nc.compile()
    return nc


_nc_cache = None


def _get_nc():
    global _nc_cache
    if _nc_cache is None:
        _nc_cache = build_nc()
    return _nc_cache


def make_in_maps(input, w, weights, biases):
    input = np.ascontiguousarray(input, dtype=np.float32)
    w = np.ascontiguousarray(w, dtype=np.float32)
    weights = np.ascontiguousarray(weights, dtype=np.float32)
    biases = np.ascontiguousarray(biases, dtype=np.float32)

    inputT = np.ascontiguousarray(input.T)                       # [IN, B]
    weightsT = np.ascontiguousarray(weights.transpose(0, 2, 1))  # [D, IN, OUT]

    in_maps = []
    for c in range(N_CORES):
        sl = slice(c * BS, (c + 1) * BS)
        in_maps.append({
            "inputT": np.ascontiguousarray(inputT[:, sl]),
            "weightsT": weightsT,
            "w": np.ascontiguousarray(w[sl]),
        })
    return in_maps


def kernel(input, w, weights, biases):
    in_maps = make_in_maps(input, w, weights, biases)
    res = run_bass_kernel_spmd(_get_nc(), in_maps, list(range(N_CORES)))
    dev = np.concatenate(
        [res.results[c]["out"] for c in range(N_CORES)], axis=0
    ).astype(np.float32)
    # Bias term (0.1% of the FLOPs) added on host.
    wb = np.asarray(w, dtype=np.float32) @ np.asarray(biases, dtype=np.float32)
    return dev + wb


if __name__ == "__main__":
    rng = np.random.default_rng(0)
    inputs = {
        "input": rng.standard_normal((B, IN), dtype=np.float32),
        "w": rng.random((B, D), dtype=np.float32),
        "weights": ((rng.random((D, OUT, IN), dtype=np.float32) - 0.5) / 16.0),
        "biases": ((rng.random((D, OUT), dtype=np.float32) - 0.5) / 16.0),
    }
    got = kernel(**inputs)
    tmp = np.einsum("bi,doi->bdo", inputs["input"], inputs["weights"])
    want = np.einsum("bdo,bd->bo", tmp, inputs["w"]) + inputs["w"] @ inputs["biases"]
    err = np.abs(got - want).max() / np.abs(want).max()
    print("rel err:", err)
